# revision 30
# baseline (speedup 1.0000x reference)
"""GCNConv Trainium2 kernel: out = segment_sum(features[src], dst) @ W + b.

Strategy (8 NeuronCores, graph partitioned by destination node):
  - Host: partition the 391 dst-node tiles (128 nodes each) across 8 cores
    (LPT balance by edge count).  Edges live with their dst tile.  Features
    are replicated to every core in bf16, split into two 25000-row tables so
    gather indices fit in int16 (dma_gather requirement).
  - Device (per core): dma_gather edge source rows (bf16, 256B rows) in
    groups, emitted eagerly in consumption order and spread over all 4 SWDGE
    queues so descriptor generation and the SDMA drain overlap deeply.  Per
    (slot, stream) segment one broadcast tensor_tensor builds all one-hot
    chunks at once (bf16, never enters DVE 2-port mode so it cannot stall
    SWDGE desc-gen); per 128-edge chunk one matmul accumulates msgs.T @
    onehot into PSUM, yielding agg.T per node tile; then out.T = W.T @ agg.T
    on the TensorEngine and a fused bias-add on the Scalar engine; DMA out.T
    tiles to DRAM.
  - Host: transpose + scatter per-core tile outputs back to [50000, 128].
"""

import os
import sys

for _p in ("/opt/trn_rl_repo",):
    if _p not in sys.path and os.path.isdir(_p):
        sys.path.insert(0, _p)

import numpy as np
import ml_dtypes

P = 128
N_NODES = 50000
N_EDGES = 640000
D = 128
NCORES = 8
HALF = 25000          # int16 index-range split of the feature table
NTILE = (N_NODES + P - 1) // P          # 391
NSLOT = (NTILE + NCORES - 1) // NCORES  # 49 node tiles per core
GCHUNK = 16           # chunks (of 128 gathered rows) per dma_gather call
NQUEUES = 4           # SWDGE queues; gather desc-gen contexts run concurrently
GBUFS = 8             # in-flight gather buffers per stream
SINGLE_PACKET = False

BF16 = ml_dtypes.bfloat16


# ---------------------------------------------------------------- host plan

def plan(src, dst):
    """Pack nodes into custom (core, slot) tiles of <=128 nodes so that each
    tile's lo/hi edge counts land just under chunk (128-edge) boundaries and
    cores are balanced; then lay out padded, chunked edge lists.  Chunk
    counts are shared across cores (max over cores) so the single SPMD
    program fits every core."""
    src = np.asarray(src).astype(np.int64)
    dst = np.asarray(dst).astype(np.int64)
    d_lo = np.bincount(dst[src < HALF], minlength=N_NODES).astype(np.int64)
    d_hi = np.bincount(dst[src >= HALF], minlength=N_NODES).astype(np.int64)
    d = d_lo + d_hi

    # Phase 1: nodes -> cores (LPT on total degree, node-count cap)
    order = np.argsort(-d, kind="stable")
    core_tot = np.zeros(NCORES)
    core_n = np.zeros(NCORES, int)
    core_of = np.empty(N_NODES, int)
    cap = NSLOT * P
    for n in order:
        c = min((c for c in range(NCORES) if core_n[c] < cap),
                key=lambda c: core_tot[c])
        core_of[n] = c
        core_tot[c] += d[n]
        core_n[c] += 1

    # Phase 2: common per-slot chunk schedule (shared across cores)
    SLACK = 3
    lo_tot = np.bincount(core_of, weights=d_lo, minlength=NCORES)
    hi_tot = np.bincount(core_of, weights=d_hi, minlength=NCORES)
    KA = int(np.ceil(lo_tot.max() / P)) + SLACK
    KB = int(np.ceil(hi_tot.max() / P)) + SLACK

    def distribute(K, slots):
        base, extra = K // slots, K - (K // slots) * slots
        return np.array([base + 1] * extra + [base] * (slots - extra))

    A = distribute(KA, NSLOT)
    B = distribute(KB, NSLOT)[::-1]

    # Phase 3: per core, fill slots steering (lo, hi) sums to the caps
    node_lists = [[None] * NSLOT for _ in range(NCORES)]
    Klo_all = np.zeros((NCORES, NSLOT), int)
    Khi_all = np.zeros((NCORES, NSLOT), int)
    slot_of = np.empty(N_NODES, int)
    pos_of = np.empty(N_NODES, int)
    for c in range(NCORES):
        nodes = np.where(core_of == c)[0]
        dl = d_lo[nodes].astype(np.float64)
        dh = d_hi[nodes].astype(np.float64)
        alive = np.ones(len(nodes), bool)
        for s in range(NSLOT):
            TL, TH = A[s] * P, B[s] * P
            lo = hi = 0.0
            rn = P
            take = []
            idxs = np.where(alive)[0]
            for _ in range(min(P, int(alive.sum()))):
                idxs = idxs[alive[idxs]]
                if len(idxs) == 0:
                    break
                bl, bh = (TL - lo) / rn, (TH - hi) / rn
                ok = (dl[idxs] <= TL - lo) & (dh[idxs] <= TH - hi)
                cand = idxs[ok] if ok.any() else idxs
                pick = cand[np.argmin(np.abs(dl[cand] - bl) +
                                      np.abs(dh[cand] - bh))]
                take.append(pick)
                alive[pick] = False
                lo += dl[pick]
                hi += dh[pick]
                rn -= 1
                if rn == 0:
                    break
            ns = nodes[np.array(take, int)] if take else np.empty(0, int)
            node_lists[c][s] = ns
            slot_of[ns] = s
            pos_of[ns] = np.arange(len(ns))
            Klo_all[c, s] = -(-int(lo) // P)
            Khi_all[c, s] = -(-int(hi) // P)

    # edges grouped by (core, slot)
    ekey = core_of[dst] * NSLOT + slot_of[dst]
    edge_order = np.argsort(ekey, kind="stable")
    ekey_sorted = ekey[edge_order]
    starts = np.searchsorted(ekey_sorted, np.arange(NCORES * NSLOT))
    ends = np.searchsorted(ekey_sorted, np.arange(NCORES * NSLOT), side="right")

    # Phase 4: block assignment.  Each source row used by a core is assigned
    # to the slot where it has the most edges; NB[s]*128 assigned rows per
    # slot are HOST-pre-gathered into a per-core "blk" image that the device
    # loads with big HWDGE DMAs (no SWDGE descriptor generation).  Each block
    # lane covers one edge of its row; every other edge goes through the
    # per-edge dma_gather streams.
    nass = np.zeros((NCORES, NSLOT), int)
    assign = []  # per core: dict slot -> assigned row array (count-desc order)
    for c in range(NCORES):
        e_c = edge_order[starts[c * NSLOT]:ends[(c + 1) * NSLOT - 1]]
        es, ed = src[e_c], dst[e_c]
        key = es * NSLOT + slot_of[ed]
        ukey, cnts = np.unique(key, return_counts=True)
        rows, slots = ukey // NSLOT, ukey % NSLOT
        o = np.lexsort((-cnts, rows))
        rows_o, slots_o, cnts_o = rows[o], slots[o], cnts[o]
        firstm = np.concatenate([[True], rows_o[1:] != rows_o[:-1]])
        arow, aslot, acnt = rows_o[firstm], slots_o[firstm], cnts_o[firstm]
        per = {}
        for s in range(NSLOT):
            sel = aslot == s
            ar, ac = arow[sel], acnt[sel]
            oo = np.argsort(-ac, kind="stable")
            per[s] = ar[oo]
            nass[c, s] = len(ar)
        assign.append(per)
    NB = (nass // P).min(axis=0)
    NBt = int(NB.sum())
    nbbase = np.concatenate([[0], np.cumsum(NB)])[:NSLOT]

    # per (core, slot): block rows + covered-edge dstl, residual edge lists
    blk_rows = np.zeros((NCORES, NBt * P), np.int64)
    Klo_r = np.zeros((NCORES, NSLOT), int)
    Khi_r = np.zeros((NCORES, NSLOT), int)
    resid = [[None] * NSLOT for _ in range(NCORES)]  # (lo_edges, hi_edges)
    blk_dstl = np.full((NCORES, NBt, P), -1.0, np.float32)
    for c in range(NCORES):
        for s in range(NSLOT):
            e_all = edge_order[starts[c * NSLOT + s]:ends[c * NSLOT + s]]
            rows_e = src[e_all]
            take = assign[c][s][:NB[s] * P]
            blk_rows[c, nbbase[s] * P:(nbbase[s] + NB[s]) * P] = take
            # cover the first edge of each block row
            o = np.argsort(rows_e, kind="stable")
            ur, fidx = np.unique(rows_e[o], return_index=True)
            first_of = dict(zip(ur.tolist(), e_all[o][fidx].tolist()))
            cov = np.array([first_of[r] for r in take], np.int64)
            for j, e in enumerate(cov):
                blk_dstl[c, nbbase[s] + j // P, j % P] = pos_of[dst[e]]
            mask = np.ones(len(e_all), bool)
            covset = set(cov.tolist())
            for i, e in enumerate(e_all):
                if e in covset:
                    mask[i] = False
                    covset.remove(e)
            e_res = e_all[mask]
            mlo = src[e_res] < HALF
            resid[c][s] = (e_res[mlo], e_res[~mlo])
            Klo_r[c, s] = -(-len(resid[c][s][0]) // P)
            Khi_r[c, s] = -(-len(resid[c][s][1]) // P)

    Klo = Klo_r.max(axis=0)
    Khi = Khi_r.max(axis=0)
    for s in range(NSLOT):  # every slot needs >=1 chunk so PSUM is written
        if NB[s] + Klo[s] + Khi[s] == 0:
            Klo[s] = 1
    KLO, KHI = int(Klo.sum()), int(Khi.sum())
    lo_base = np.concatenate([[0], np.cumsum(Klo)])[:NSLOT]
    hi_base = np.concatenate([[0], np.cumsum(Khi)])[:NSLOT]

    # per-core padded gather index / dst_local arrays, chunk-major
    # dstl columns: [blk (NBt) | lo (KLO) | hi (KHI)]
    idx = np.zeros((NCORES, KLO + KHI, P), np.int16)
    dstl = np.full((NCORES, NBt + KLO + KHI, P), -1.0, np.float32)
    dstl[:, :NBt, :] = blk_dstl
    for c in range(NCORES):
        for s in range(NSLOT):
            for K, bases, e, stream_off, table_off in (
                (Klo[s], lo_base, resid[c][s][0], 0, 0),
                (Khi[s], hi_base, resid[c][s][1], KLO, HALF),
            ):
                if K == 0:
                    continue
                b0 = stream_off + bases[s]
                flat_i = idx[c, b0:b0 + K].reshape(-1)
                flat_d = dstl[c, NBt + b0:NBt + b0 + K].reshape(-1)
                flat_i[: len(e)] = (src[e] - table_off).astype(np.int16)
                flat_d[: len(e)] = pos_of[dst[e]].astype(np.float32)

    return {
        "node_lists": node_lists,
        "NB": NB, "Klo": Klo, "Khi": Khi, "KLO": KLO, "KHI": KHI,
        "blk_rows": blk_rows,
        "idx": idx, "dstl": dstl,
    }


def _groups(K):
    """Split stream of K chunks into gather groups of <= GCHUNK chunks."""
    out = []
    c = 0
    while c < K:
        out.append((c, min(c + GCHUNK, K)))
        c = out[-1][1]
    return out


def _interleave(Klo, Khi):
    """Order lo/hi gather groups by first consumption point: walk the slots
    (lo segment then hi segment per slot) and append a stream's next group
    when the consumption cursor first enters it."""
    lo_groups, hi_groups = _groups(int(np.sum(Klo))), _groups(int(np.sum(Khi)))
    order = []
    nxt = {"lo": 0, "hi": 0}
    cur = {"lo": 0, "hi": 0}
    groups = {"lo": lo_groups, "hi": hi_groups}
    for s in range(len(Klo)):
        for name, k in (("lo", int(Klo[s])), ("hi", int(Khi[s]))):
            cur[name] += k
            while nxt[name] < len(groups[name]) and \
                    groups[name][nxt[name]][0] < cur[name]:
                order.append((name, groups[name][nxt[name]]))
                nxt[name] += 1
    return lo_groups, hi_groups, order


def pack_gidx(idx):
    """[K,128] int16 chunk-major indices -> [128, K*8] dma_gather layout
    (index i of a group at [i%16, i//16], replicated on partitions 16..127)."""
    K = idx.shape[0]
    out = np.zeros((128, K * 8), np.int16)
    for c0, c1 in _groups(K):
        g = idx[c0:c1].reshape(-1)                # i = (c-c0)*128 + lane
        blk = g.reshape(-1, 16).T                 # [16, (c1-c0)*8]
        out[:, c0 * 8:c1 * 8] = np.tile(blk, (8, 1))
    return out


# ---------------------------------------------------------------- program

def build(NB, Klo, Khi, dbg=False):
    import concourse.bass as bass
    import concourse.mybir as mybir
    from concourse import bacc
    import concourse.tile as tile

    KLO, KHI = int(np.sum(Klo)), int(np.sum(Khi))
    NBt = int(np.sum(NB))
    NCH = NBt + KLO + KHI
    bf16, f32, i16 = mybir.dt.bfloat16, mybir.dt.float32, mybir.dt.int16

    nbbase = np.concatenate([[0], np.cumsum(NB)])[:NSLOT]
    lo_base = np.concatenate([[0], np.cumsum(Klo)])[:NSLOT]
    hi_base = np.concatenate([[0], np.cumsum(Khi)])[:NSLOT]

    nc = bacc.Bacc("TRN2", debug=dbg, num_swdge_queues=NQUEUES)
    flo = nc.dram_tensor("flo", [HALF, D], bf16, kind="ExternalInput")
    fhi = nc.dram_tensor("fhi", [N_NODES - HALF, D], bf16, kind="ExternalInput")
    blk = nc.dram_tensor("blk", [P, NBt * P], bf16, kind="ExternalInput")
    gidx = nc.dram_tensor("gidx", [P, (KLO + KHI) * 8], i16, kind="ExternalInput")
    dstl = nc.dram_tensor("dstl", [P, NCH], bf16, kind="ExternalInput")
    iota = nc.dram_tensor("iota", [P, P], bf16, kind="ExternalInput")
    wmat = nc.dram_tensor("wmat", [P, P], bf16, kind="ExternalInput")
    bcol = nc.dram_tensor("bcol", [P, 1], f32, kind="ExternalInput")
    out = nc.dram_tensor("out", [P, NSLOT * P], f32, kind="ExternalOutput")

    lo_groups, hi_groups, gorder = _interleave(Klo, Khi)
    KSEG = int(max(Klo.max(), Khi.max(), NB.max(), 1))

    with tile.TileContext(nc) as tc:
        with tc.tile_pool(name="const", bufs=1) as cp, \
             tc.tile_pool(name="glo", bufs=GBUFS) as gplo, \
             tc.tile_pool(name="ghi", bufs=GBUFS) as gphi, \
             tc.tile_pool(name="oh", bufs=6) as ohp, \
             tc.tile_pool(name="res", bufs=3) as resp, \
             tc.tile_pool(name="psA", bufs=4, space="PSUM") as psA, \
             tc.tile_pool(name="psB", bufs=2, space="PSUM") as psB:

            # gather indices for the first two groups land first so the first
            # gathers can issue immediately; everything else loads behind them
            gidx_t = cp.tile([P, (KLO + KHI) * 8], i16)
            first = gorder[:2]
            done = {"lo": 0, "hi": 0}
            for name, (c0, c1) in first:
                off = 0 if name == "lo" else KLO
                nc.sync.dma_start(out=gidx_t[:, (off + c0) * 8:(off + c1) * 8],
                                  in_=gidx[:, (off + c0) * 8:(off + c1) * 8])
                done[name] = max(done[name], c1)

            st = {
                "lo": {"groups": lo_groups, "tab": flo, "pool": gplo,
                       "tiles": {}, "coff": 0, "doff": NBt, "g": 0},
                "hi": {"groups": hi_groups, "tab": fhi, "pool": gphi,
                       "tiles": {}, "coff": KLO, "doff": NBt + KLO, "g": 0},
            }

            qcount = [0]

            def fetch(name):
                S = st[name]
                gi = S["g"]
                c0, c1 = S["groups"][gi]
                n = c1 - c0
                t = S["pool"].tile([P, n * P], mybir.dt.bfloat16, tag="g" + name)
                nc.gpsimd.dma_gather(
                    out_ap=t[:].rearrange("p (g d) -> p g d", d=P),
                    in_ap=S["tab"][:],
                    idxs_ap=gidx_t[:, (S["coff"] + c0) * 8:(S["coff"] + c1) * 8],
                    num_idxs=n * P,
                    num_idxs_reg=n * P,
                    elem_size=P,
                    single_packet=SINGLE_PACKET,
                    queue_num=qcount[0] % NQUEUES,
                )
                qcount[0] += 1
                S["tiles"][gi] = (t, c0, c1)
                S["g"] += 1

            # first gathers, then the rest of the constants, then all other
            # gathers (consumption order; buffer pool depth throttles them)
            for name, _ in first:
                fetch(name)

            # remaining gidx in two bulk DMAs (lo tail, hi tail)
            for name, K in (("lo", KLO), ("hi", KHI)):
                off = 0 if name == "lo" else KLO
                c0 = done[name]
                if c0 < K:
                    nc.sync.dma_start(out=gidx_t[:, (off + c0) * 8:(off + K) * 8],
                                      in_=gidx[:, (off + c0) * 8:(off + K) * 8])
            dstl_t = cp.tile([P, NCH], bf16)
            nc.sync.dma_start(out=dstl_t[:], in_=dstl[:])
            iota_t = cp.tile([P, P], bf16)
            nc.sync.dma_start(out=iota_t[:], in_=iota[:])
            w_t = cp.tile([P, P], bf16)
            nc.sync.dma_start(out=w_t[:], in_=wmat[:])
            b_t = cp.tile([P, 1], f32)
            nc.sync.dma_start(out=b_t[:], in_=bcol[:])

            # resident block image: big HWDGE loads (no SWDGE desc-gen),
            # split so early slots' blocks land first
            blk_t = cp.tile([P, NBt * P], bf16)
            NSPLIT = 8
            bnds = [NBt * i // NSPLIT for i in range(NSPLIT + 1)]
            for i in range(NSPLIT):
                if bnds[i] < bnds[i + 1]:
                    nc.sync.dma_start(
                        out=blk_t[:, bnds[i] * P:bnds[i + 1] * P],
                        in_=blk[:, bnds[i] * P:bnds[i + 1] * P])

            for name, _ in gorder[2:]:
                fetch(name)

            def onehot(dc0, k):
                """one tensor_tensor -> [P, k*128] bf16 one-hot for k chunks
                whose dstl columns start at global chunk dc0."""
                oh = ohp.tile([P, KSEG * P], mybir.dt.bfloat16, tag="oh")
                in0 = iota_t[:].rearrange("p (k f) -> p k f", k=1) \
                    .broadcast_to([P, k, P])
                in1 = dstl_t[:, dc0:dc0 + k].rearrange("p (k o) -> p k o", o=1) \
                    .broadcast_to([P, k, P])
                outv = oh[:, :k * P].rearrange("p (k f) -> p k f", k=k)
                nc.vector.tensor_tensor(out=outv, in0=in0, in1=in1,
                                        op=mybir.AluOpType.is_equal)
                return oh

            cur = {"lo": 0, "hi": 0}  # global chunk cursor per stream
            gcur = {"lo": 0, "hi": 0}  # current group idx per stream
            for s in range(NSLOT):
                nch = int(NB[s] + Klo[s] + Khi[s])
                ps_agg = psA.tile([P, P], f32, tag="agg")
                ci = 0
                if NB[s] > 0:
                    oh = onehot(int(nbbase[s]), int(NB[s]))
                    for b in range(int(NB[s])):
                        col = (int(nbbase[s]) + b) * P
                        nc.tensor.matmul(
                            out=ps_agg[:],
                            lhsT=blk_t[:, col:col + P],
                            rhs=oh[:, b * P:(b + 1) * P],
                            start=(ci == 0), stop=(ci == nch - 1),
                        )
                        ci += 1
                for name, k, sbase in (("lo", int(Klo[s]), int(lo_base[s])),
                                       ("hi", int(Khi[s]), int(hi_base[s]))):
                    if k == 0:
                        continue
                    S = st[name]
                    oh = onehot(S["doff"] + sbase, k)
                    for j in range(k):
                        pos = cur[name]
                        while pos >= S["groups"][gcur[name]][1]:
                            gcur[name] += 1
                        t, c0, c1 = S["tiles"][gcur[name]]
                        off = pos - c0
                        nc.tensor.matmul(
                            out=ps_agg[:],
                            lhsT=t[:, off * P:(off + 1) * P],
                            rhs=oh[:, j * P:(j + 1) * P],
                            start=(ci == 0), stop=(ci == nch - 1),
                        )
                        cur[name] += 1
                        ci += 1

                aggT = resp.tile([P, P], mybir.dt.bfloat16, tag="aggT")
                nc.scalar.copy(out=aggT[:], in_=ps_agg[:])
                ps_out = psB.tile([P, P], f32, tag="out")
                nc.tensor.matmul(out=ps_out[:], lhsT=w_t[:], rhs=aggT[:],
                                 start=True, stop=True)
                o_sb = resp.tile([P, P], f32, tag="osb")
                nc.scalar.activation(
                    out=o_sb[:], in_=ps_out[:],
                    func=mybir.ActivationFunctionType.Identity,
                    bias=b_t[:, 0:1],
                )
                nc.sync.dma_start(out=out[:, s * P:(s + 1) * P], in_=o_sb[:])

    # Spread gathers across SWDGE queues.  Tile assigns each Pool-engine DMA
    # a DMASW completion lane in *scheduled* order; queue choice must be a
    # function of that lane (the sim/ucode bind each lane to one queue), so
    # retag after scheduling: queue = lane % NQUEUES.
    for inst in nc.inst_map.values():
        if isinstance(inst, mybir.InstDMAGatherAnt):
            proc = inst.bass_scheduled_proc
            if proc is not None and 11 <= proc <= 18:
                inst.queue_num = (proc - 11) % NQUEUES

    nc.compile()
    return nc


# ---------------------------------------------------------------- in_maps

def make_in_maps(features, W, b, pl):
    f16 = np.ascontiguousarray(features).astype(BF16)
    iota_np = np.tile(np.arange(P, dtype=np.float32)[None, :], (P, 1)).astype(BF16)
    w_np = np.asarray(W, np.float32).astype(BF16)
    b_np = np.asarray(b, np.float32).reshape(1, D).T.copy()  # [128,1]
    NBt = int(pl["NB"].sum())
    in_maps = []
    for c in range(NCORES):
        # blk image: partition = lane, free = (chunk, feat); host pre-gather
        rows = pl["blk_rows"][c]
        blk = f16[rows].reshape(NBt, P, D).transpose(1, 0, 2) \
            .reshape(P, NBt * D).copy()
        in_maps.append({
            "flo": f16[:HALF],
            "fhi": f16[HALF:],
            "blk": blk,
            "gidx": pack_gidx(pl["idx"][c]),
            "dstl": np.ascontiguousarray(pl["dstl"][c].T).astype(BF16),
            "iota": iota_np,
            "wmat": w_np,
            "bcol": b_np,
        })
    return in_maps


def unshard(outs, node_lists):
    """outs: list of {'out': [128, NSLOT*128] f32} per core -> [50000,128]."""
    full = np.zeros((N_NODES, D), np.float32)
    for c in range(NCORES):
        oT = np.asarray(outs[c]["out"], np.float32)
        for s in range(NSLOT):
            ns = node_lists[c][s]
            if len(ns) == 0:
                continue
            full[ns, :] = oT[:, s * P:s * P + len(ns)].T
    return full


# ---------------------------------------------------------------- entry

_CACHE = {}


def kernel(features, src, dst, W, b):
    from concourse.bass_utils import run_bass_kernel_spmd

    pl = plan(src, dst)
    key = (tuple(pl["NB"]), tuple(pl["Klo"]), tuple(pl["Khi"]))
    if key not in _CACHE:
        _CACHE[key] = build(pl["NB"], pl["Klo"], pl["Khi"])
    nc = _CACHE[key]
    in_maps = make_in_maps(features, W, b, pl)
    last = None
    for _ in range(3):  # retry: a previously wedged pool device can fail a load
        try:
            res = run_bass_kernel_spmd(nc, in_maps, core_ids=list(range(NCORES)))
            return unshard(res.results, pl["node_lists"])
        except Exception as e:  # noqa: BLE001
            last = e
    raise last


# revision 38
# speedup vs baseline: 1.4736x; 1.4736x over previous
"""GCNConv Trainium2 kernel: out = segment_sum(features[src], dst) @ W + b.

Strategy (8 NeuronCores, graph partitioned by destination node):
  - Host: partition the 391 dst-node tiles (128 nodes each) across 8 cores
    (LPT balance by edge count).  Edges live with their dst tile.  Features
    are replicated to every core in bf16, split into two 25000-row tables so
    gather indices fit in int16 (dma_gather requirement).
  - Device (per core): dma_gather edge source rows (bf16, 256B rows) in
    groups, emitted eagerly in consumption order and spread over all 4 SWDGE
    queues so descriptor generation and the SDMA drain overlap deeply.  Per
    (slot, stream) segment one broadcast tensor_tensor builds all one-hot
    chunks at once (bf16, never enters DVE 2-port mode so it cannot stall
    SWDGE desc-gen); per 128-edge chunk one matmul accumulates msgs.T @
    onehot into PSUM, yielding agg.T per node tile; then out.T = W.T @ agg.T
    on the TensorEngine and a fused bias-add on the Scalar engine; DMA out.T
    tiles to DRAM.
  - Host: transpose + scatter per-core tile outputs back to [50000, 128].
"""

import os
import sys

for _p in ("/opt/trn_rl_repo",):
    if _p not in sys.path and os.path.isdir(_p):
        sys.path.insert(0, _p)

import numpy as np
import ml_dtypes

P = 128
N_NODES = 50000
N_EDGES = 640000
D = 128
NCORES = 8
HALF = 25000          # int16 index-range split of the feature table
NTILE = (N_NODES + P - 1) // P          # 391
NSLOT = (NTILE + NCORES - 1) // NCORES  # 49 node tiles per core
GCHUNK = 16           # chunks (of 128 gathered rows) per dma_gather call
NQUEUES = 4           # SWDGE queues; gather desc-gen contexts run concurrently
GBUFS = 8             # in-flight gather buffers per stream
SINGLE_PACKET = False

BF16 = ml_dtypes.bfloat16


# ---------------------------------------------------------------- host plan

def plan(src, dst):
    """Pack nodes into custom (core, slot) tiles of <=128 nodes so that each
    tile's lo/hi edge counts land just under chunk (128-edge) boundaries and
    cores are balanced; then lay out padded, chunked edge lists.  Chunk
    counts are shared across cores (max over cores) so the single SPMD
    program fits every core."""
    src = np.asarray(src).astype(np.int64)
    dst = np.asarray(dst).astype(np.int64)
    d_lo = np.bincount(dst[src < HALF], minlength=N_NODES).astype(np.int64)
    d_hi = np.bincount(dst[src >= HALF], minlength=N_NODES).astype(np.int64)
    d = d_lo + d_hi

    # Phase 1: nodes -> cores (LPT on total degree, node-count cap)
    order = np.argsort(-d, kind="stable")
    core_tot = np.zeros(NCORES)
    core_n = np.zeros(NCORES, int)
    core_of = np.empty(N_NODES, int)
    cap = NSLOT * P
    for n in order:
        c = min((c for c in range(NCORES) if core_n[c] < cap),
                key=lambda c: core_tot[c])
        core_of[n] = c
        core_tot[c] += d[n]
        core_n[c] += 1

    # Phase 2: common per-slot chunk schedule (shared across cores)
    SLACK = 3
    lo_tot = np.bincount(core_of, weights=d_lo, minlength=NCORES)
    hi_tot = np.bincount(core_of, weights=d_hi, minlength=NCORES)
    KA = int(np.ceil(lo_tot.max() / P)) + SLACK
    KB = int(np.ceil(hi_tot.max() / P)) + SLACK

    def distribute(K, slots):
        base, extra = K // slots, K - (K // slots) * slots
        return np.array([base + 1] * extra + [base] * (slots - extra))

    A = distribute(KA, NSLOT)
    B = distribute(KB, NSLOT)[::-1]

    # Phase 3: per core, fill slots steering (lo, hi) sums to the caps
    node_lists = [[None] * NSLOT for _ in range(NCORES)]
    Klo_all = np.zeros((NCORES, NSLOT), int)
    Khi_all = np.zeros((NCORES, NSLOT), int)
    slot_of = np.empty(N_NODES, int)
    pos_of = np.empty(N_NODES, int)
    for c in range(NCORES):
        nodes = np.where(core_of == c)[0]
        dl = d_lo[nodes].astype(np.float64)
        dh = d_hi[nodes].astype(np.float64)
        alive = np.ones(len(nodes), bool)
        for s in range(NSLOT):
            TL, TH = A[s] * P, B[s] * P
            lo = hi = 0.0
            rn = P
            take = []
            idxs = np.where(alive)[0]
            for _ in range(min(P, int(alive.sum()))):
                idxs = idxs[alive[idxs]]
                if len(idxs) == 0:
                    break
                bl, bh = (TL - lo) / rn, (TH - hi) / rn
                ok = (dl[idxs] <= TL - lo) & (dh[idxs] <= TH - hi)
                cand = idxs[ok] if ok.any() else idxs
                pick = cand[np.argmin(np.abs(dl[cand] - bl) +
                                      np.abs(dh[cand] - bh))]
                take.append(pick)
                alive[pick] = False
                lo += dl[pick]
                hi += dh[pick]
                rn -= 1
                if rn == 0:
                    break
            ns = nodes[np.array(take, int)] if take else np.empty(0, int)
            node_lists[c][s] = ns
            slot_of[ns] = s
            pos_of[ns] = np.arange(len(ns))
            Klo_all[c, s] = -(-int(lo) // P)
            Khi_all[c, s] = -(-int(hi) // P)

    # edges grouped by (core, slot)
    ekey = core_of[dst] * NSLOT + slot_of[dst]
    edge_order = np.argsort(ekey, kind="stable")
    ekey_sorted = ekey[edge_order]
    starts = np.searchsorted(ekey_sorted, np.arange(NCORES * NSLOT))
    ends = np.searchsorted(ekey_sorted, np.arange(NCORES * NSLOT), side="right")

    # Phase 4: block assignment.  Each source row used by a core is assigned
    # to the slot where it has the most edges; NB[s]*128 assigned rows per
    # slot are HOST-pre-gathered into a per-core "blk" image that the device
    # loads with big HWDGE DMAs (no SWDGE descriptor generation).  Each block
    # lane covers one edge of its row; every other edge goes through the
    # per-edge dma_gather streams.
    nass = np.zeros((NCORES, NSLOT), int)
    assign = []  # per core: dict slot -> assigned row array (count-desc order)
    for c in range(NCORES):
        e_c = edge_order[starts[c * NSLOT]:ends[(c + 1) * NSLOT - 1]]
        es, ed = src[e_c], dst[e_c]
        key = es * NSLOT + slot_of[ed]
        ukey, cnts = np.unique(key, return_counts=True)
        rows, slots = ukey // NSLOT, ukey % NSLOT
        o = np.lexsort((-cnts, rows))
        rows_o, slots_o, cnts_o = rows[o], slots[o], cnts[o]
        firstm = np.concatenate([[True], rows_o[1:] != rows_o[:-1]])
        arow, aslot, acnt = rows_o[firstm], slots_o[firstm], cnts_o[firstm]
        per = {}
        for s in range(NSLOT):
            sel = aslot == s
            ar, ac = arow[sel], acnt[sel]
            oo = np.argsort(-ac, kind="stable")
            per[s] = ar[oo]
            nass[c, s] = len(ar)
        assign.append(per)
    NB = (nass // P).min(axis=0)
    NBt = int(NB.sum())
    nbbase = np.concatenate([[0], np.cumsum(NB)])[:NSLOT]

    # per (core, slot): block rows + covered-edge dstl, residual edge lists
    blk_rows = np.zeros((NCORES, NBt * P), np.int64)
    Klo_r = np.zeros((NCORES, NSLOT), int)
    Khi_r = np.zeros((NCORES, NSLOT), int)
    resid = [[None] * NSLOT for _ in range(NCORES)]  # (lo_edges, hi_edges)
    blk_dstl = np.full((NCORES, NBt, P), -1.0, np.float32)
    for c in range(NCORES):
        for s in range(NSLOT):
            e_all = edge_order[starts[c * NSLOT + s]:ends[c * NSLOT + s]]
            rows_e = src[e_all]
            take = assign[c][s][:NB[s] * P]
            blk_rows[c, nbbase[s] * P:(nbbase[s] + NB[s]) * P] = take
            # cover the first edge of each block row
            o = np.argsort(rows_e, kind="stable")
            ur, fidx = np.unique(rows_e[o], return_index=True)
            first_of = dict(zip(ur.tolist(), e_all[o][fidx].tolist()))
            cov = np.array([first_of[r] for r in take], np.int64)
            for j, e in enumerate(cov):
                blk_dstl[c, nbbase[s] + j // P, j % P] = pos_of[dst[e]]
            mask = np.ones(len(e_all), bool)
            covset = set(cov.tolist())
            for i, e in enumerate(e_all):
                if e in covset:
                    mask[i] = False
                    covset.remove(e)
            e_res = e_all[mask]
            mlo = src[e_res] < HALF
            resid[c][s] = (e_res[mlo], e_res[~mlo])
            Klo_r[c, s] = -(-len(resid[c][s][0]) // P)
            Khi_r[c, s] = -(-len(resid[c][s][1]) // P)

    Klo = Klo_r.max(axis=0)
    Khi = Khi_r.max(axis=0)
    for s in range(NSLOT):  # every slot needs >=1 chunk so PSUM is written
        if NB[s] + Klo[s] + Khi[s] == 0:
            Klo[s] = 1
    KLO, KHI = int(Klo.sum()), int(Khi.sum())
    lo_base = np.concatenate([[0], np.cumsum(Klo)])[:NSLOT]
    hi_base = np.concatenate([[0], np.cumsum(Khi)])[:NSLOT]

    # per-core padded gather index / dst_local arrays, chunk-major
    # dstl columns: [blk (NBt) | lo (KLO) | hi (KHI)]
    idx = np.zeros((NCORES, KLO + KHI, P), np.int16)
    dstl = np.full((NCORES, NBt + KLO + KHI, P), -1.0, np.float32)
    dstl[:, :NBt, :] = blk_dstl
    for c in range(NCORES):
        for s in range(NSLOT):
            for K, bases, e, stream_off, table_off in (
                (Klo[s], lo_base, resid[c][s][0], 0, 0),
                (Khi[s], hi_base, resid[c][s][1], KLO, HALF),
            ):
                if K == 0:
                    continue
                b0 = stream_off + bases[s]
                flat_i = idx[c, b0:b0 + K].reshape(-1)
                flat_d = dstl[c, NBt + b0:NBt + b0 + K].reshape(-1)
                flat_i[: len(e)] = (src[e] - table_off).astype(np.int16)
                flat_d[: len(e)] = pos_of[dst[e]].astype(np.float32)

    return {
        "node_lists": node_lists,
        "NB": NB, "Klo": Klo, "Khi": Khi, "KLO": KLO, "KHI": KHI,
        "blk_rows": blk_rows,
        "idx": idx, "dstl": dstl,
    }


def _groups(K):
    """Split stream of K chunks into gather groups of <= GCHUNK chunks."""
    out = []
    c = 0
    while c < K:
        out.append((c, min(c + GCHUNK, K)))
        c = out[-1][1]
    return out


def _interleave(Klo, Khi):
    """Order lo/hi gather groups by first consumption point: walk the slots
    (lo segment then hi segment per slot) and append a stream's next group
    when the consumption cursor first enters it."""
    lo_groups, hi_groups = _groups(int(np.sum(Klo))), _groups(int(np.sum(Khi)))
    order = []
    nxt = {"lo": 0, "hi": 0}
    cur = {"lo": 0, "hi": 0}
    groups = {"lo": lo_groups, "hi": hi_groups}
    for s in range(len(Klo)):
        for name, k in (("lo", int(Klo[s])), ("hi", int(Khi[s]))):
            cur[name] += k
            while nxt[name] < len(groups[name]) and \
                    groups[name][nxt[name]][0] < cur[name]:
                order.append((name, groups[name][nxt[name]]))
                nxt[name] += 1
    return lo_groups, hi_groups, order


def pack_gidx(idx):
    """[K,128] int16 chunk-major indices -> [128, K*8] dma_gather layout
    (index i of a group at [i%16, i//16], replicated on partitions 16..127)."""
    K = idx.shape[0]
    out = np.zeros((128, K * 8), np.int16)
    for c0, c1 in _groups(K):
        g = idx[c0:c1].reshape(-1)                # i = (c-c0)*128 + lane
        blk = g.reshape(-1, 16).T                 # [16, (c1-c0)*8]
        out[:, c0 * 8:c1 * 8] = np.tile(blk, (8, 1))
    return out


# ---------------------------------------------------------------- program

def build(NB, Klo, Khi, dbg=False):
    import concourse.bass as bass
    import concourse.mybir as mybir
    from concourse import bacc
    import concourse.tile as tile

    KLO, KHI = int(np.sum(Klo)), int(np.sum(Khi))
    NBt = int(np.sum(NB))
    NCH = NBt + KLO + KHI
    bf16, f32, i16 = mybir.dt.bfloat16, mybir.dt.float32, mybir.dt.int16

    nbbase = np.concatenate([[0], np.cumsum(NB)])[:NSLOT]
    lo_base = np.concatenate([[0], np.cumsum(Klo)])[:NSLOT]
    hi_base = np.concatenate([[0], np.cumsum(Khi)])[:NSLOT]

    nc = bacc.Bacc("TRN2", debug=dbg, num_swdge_queues=NQUEUES)
    flo = nc.dram_tensor("flo", [HALF, D], bf16, kind="ExternalInput")
    fhi = nc.dram_tensor("fhi", [N_NODES - HALF, D], bf16, kind="ExternalInput")
    blk = nc.dram_tensor("blk", [P, NBt * P], bf16, kind="ExternalInput")
    gidx = nc.dram_tensor("gidx", [P, (KLO + KHI) * 8], i16, kind="ExternalInput")
    dstl = nc.dram_tensor("dstl", [P, NCH], bf16, kind="ExternalInput")
    iota = nc.dram_tensor("iota", [P, P], bf16, kind="ExternalInput")
    wmat = nc.dram_tensor("wmat", [P, P], bf16, kind="ExternalInput")
    bcol = nc.dram_tensor("bcol", [P, 1], f32, kind="ExternalInput")
    out = nc.dram_tensor("out", [P, NSLOT * P], f32, kind="ExternalOutput")

    lo_groups, hi_groups, gorder = _interleave(Klo, Khi)
    KSEG = int(max(Klo.max(), Khi.max(), NB.max(), 1))

    with tile.TileContext(nc) as tc:
        with tc.tile_pool(name="const", bufs=1) as cp, \
             tc.tile_pool(name="glo", bufs=GBUFS) as gplo, \
             tc.tile_pool(name="ghi", bufs=GBUFS) as gphi, \
             tc.tile_pool(name="oh", bufs=6) as ohp, \
             tc.tile_pool(name="res", bufs=3) as resp, \
             tc.tile_pool(name="psA", bufs=4, space="PSUM") as psA, \
             tc.tile_pool(name="psB", bufs=2, space="PSUM") as psB:

            # Small consts first, then the WHOLE resident block image via big
            # HWDGE DMAs, then the gather indices.  The sync engine's HWDGE
            # ring is FIFO, so gidx lands only after blk is fully loaded --
            # which keeps bulk HWDGE SBUF writes from contending with SWDGE
            # descriptor-ring ports while dma_gather desc-gen runs.
            dstl_t = cp.tile([P, NCH], bf16)
            nc.sync.dma_start(out=dstl_t[:], in_=dstl[:])
            iota_t = cp.tile([P, P], bf16)
            nc.sync.dma_start(out=iota_t[:], in_=iota[:])
            w_t = cp.tile([P, P], bf16)
            nc.sync.dma_start(out=w_t[:], in_=wmat[:])
            b_t = cp.tile([P, 1], f32)
            nc.sync.dma_start(out=b_t[:], in_=bcol[:])

            blk_t = cp.tile([P, max(NBt, 1) * P], bf16)
            NSPLIT = 4
            bnds = [NBt * i // NSPLIT for i in range(NSPLIT + 1)]
            for i in range(NSPLIT):
                if bnds[i] < bnds[i + 1]:
                    nc.sync.dma_start(
                        out=blk_t[:, bnds[i] * P:bnds[i + 1] * P],
                        in_=blk[:, bnds[i] * P:bnds[i + 1] * P])

            gidx_t = cp.tile([P, (KLO + KHI) * 8], i16)
            first = gorder[:2]
            done = {"lo": 0, "hi": 0}
            for name, (c0, c1) in first:
                off = 0 if name == "lo" else KLO
                nc.sync.dma_start(out=gidx_t[:, (off + c0) * 8:(off + c1) * 8],
                                  in_=gidx[:, (off + c0) * 8:(off + c1) * 8])
                done[name] = max(done[name], c1)

            st = {
                "lo": {"groups": lo_groups, "tab": flo, "pool": gplo,
                       "tiles": {}, "coff": 0, "doff": NBt, "g": 0},
                "hi": {"groups": hi_groups, "tab": fhi, "pool": gphi,
                       "tiles": {}, "coff": KLO, "doff": NBt + KLO, "g": 0},
            }

            qcount = [0]

            def fetch(name):
                S = st[name]
                gi = S["g"]
                c0, c1 = S["groups"][gi]
                n = c1 - c0
                t = S["pool"].tile([P, n * P], mybir.dt.bfloat16, tag="g" + name)
                nc.gpsimd.dma_gather(
                    out_ap=t[:].rearrange("p (g d) -> p g d", d=P),
                    in_ap=S["tab"][:],
                    idxs_ap=gidx_t[:, (S["coff"] + c0) * 8:(S["coff"] + c1) * 8],
                    num_idxs=n * P,
                    num_idxs_reg=n * P,
                    elem_size=P,
                    single_packet=SINGLE_PACKET,
                    queue_num=qcount[0] % NQUEUES,
                )
                qcount[0] += 1
                S["tiles"][gi] = (t, c0, c1)
                S["g"] += 1

            # first gathers, then the rest of the constants, then all other
            # gathers (consumption order; buffer pool depth throttles them)
            for name, _ in first:
                fetch(name)

            # remaining gidx in two bulk DMAs (lo tail, hi tail)
            for name, K in (("lo", KLO), ("hi", KHI)):
                off = 0 if name == "lo" else KLO
                c0 = done[name]
                if c0 < K:
                    nc.sync.dma_start(out=gidx_t[:, (off + c0) * 8:(off + K) * 8],
                                      in_=gidx[:, (off + c0) * 8:(off + K) * 8])

            for name, _ in gorder[2:]:
                fetch(name)

            def onehot(dc0, k):
                """one tensor_tensor -> [P, k*128] bf16 one-hot for k chunks
                whose dstl columns start at global chunk dc0."""
                oh = ohp.tile([P, KSEG * P], mybir.dt.bfloat16, tag="oh")
                in0 = iota_t[:].rearrange("p (k f) -> p k f", k=1) \
                    .broadcast_to([P, k, P])
                in1 = dstl_t[:, dc0:dc0 + k].rearrange("p (k o) -> p k o", o=1) \
                    .broadcast_to([P, k, P])
                outv = oh[:, :k * P].rearrange("p (k f) -> p k f", k=k)
                nc.vector.tensor_tensor(out=outv, in0=in0, in1=in1,
                                        op=mybir.AluOpType.is_equal)
                return oh

            cur = {"lo": 0, "hi": 0}  # global chunk cursor per stream
            gcur = {"lo": 0, "hi": 0}  # current group idx per stream
            for s in range(NSLOT):
                nch = int(NB[s] + Klo[s] + Khi[s])
                ps_agg = psA.tile([P, P], f32, tag="agg")
                ci = 0
                if NB[s] > 0:
                    oh = onehot(int(nbbase[s]), int(NB[s]))
                    for b in range(int(NB[s])):
                        col = (int(nbbase[s]) + b) * P
                        nc.tensor.matmul(
                            out=ps_agg[:],
                            lhsT=blk_t[:, col:col + P],
                            rhs=oh[:, b * P:(b + 1) * P],
                            start=(ci == 0), stop=(ci == nch - 1),
                        )
                        ci += 1
                for name, k, sbase in (("lo", int(Klo[s]), int(lo_base[s])),
                                       ("hi", int(Khi[s]), int(hi_base[s]))):
                    if k == 0:
                        continue
                    S = st[name]
                    oh = onehot(S["doff"] + sbase, k)
                    for j in range(k):
                        pos = cur[name]
                        while pos >= S["groups"][gcur[name]][1]:
                            gcur[name] += 1
                        t, c0, c1 = S["tiles"][gcur[name]]
                        off = pos - c0
                        nc.tensor.matmul(
                            out=ps_agg[:],
                            lhsT=t[:, off * P:(off + 1) * P],
                            rhs=oh[:, j * P:(j + 1) * P],
                            start=(ci == 0), stop=(ci == nch - 1),
                        )
                        cur[name] += 1
                        ci += 1

                aggT = resp.tile([P, P], mybir.dt.bfloat16, tag="aggT")
                nc.scalar.copy(out=aggT[:], in_=ps_agg[:])
                ps_out = psB.tile([P, P], f32, tag="out")
                nc.tensor.matmul(out=ps_out[:], lhsT=w_t[:], rhs=aggT[:],
                                 start=True, stop=True)
                o_sb = resp.tile([P, P], f32, tag="osb")
                nc.scalar.activation(
                    out=o_sb[:], in_=ps_out[:],
                    func=mybir.ActivationFunctionType.Identity,
                    bias=b_t[:, 0:1],
                )
                nc.sync.dma_start(out=out[:, s * P:(s + 1) * P], in_=o_sb[:])

    # Spread gathers across SWDGE queues.  Tile assigns each Pool-engine DMA
    # a DMASW completion lane in *scheduled* order; queue choice must be a
    # function of that lane (the sim/ucode bind each lane to one queue), so
    # retag after scheduling: queue = lane % NQUEUES.
    for inst in nc.inst_map.values():
        if isinstance(inst, mybir.InstDMAGatherAnt):
            proc = inst.bass_scheduled_proc
            if proc is not None and 11 <= proc <= 18:
                inst.queue_num = (proc - 11) % NQUEUES

    nc.compile()
    return nc


# ---------------------------------------------------------------- in_maps

def make_in_maps(features, W, b, pl):
    f16 = np.ascontiguousarray(features).astype(BF16)
    iota_np = np.tile(np.arange(P, dtype=np.float32)[None, :], (P, 1)).astype(BF16)
    w_np = np.asarray(W, np.float32).astype(BF16)
    b_np = np.asarray(b, np.float32).reshape(1, D).T.copy()  # [128,1]
    NBt = int(pl["NB"].sum())
    in_maps = []
    for c in range(NCORES):
        # blk image: partition = lane, free = (chunk, feat); host pre-gather
        rows = pl["blk_rows"][c]
        blkimg = f16[rows].reshape(NBt, P, D).transpose(1, 0, 2) \
            .reshape(P, NBt * D).copy()
        in_maps.append({
            "flo": f16[:HALF],
            "fhi": f16[HALF:],
            "blk": blkimg,
            "gidx": pack_gidx(pl["idx"][c]),
            "dstl": np.ascontiguousarray(pl["dstl"][c].T).astype(BF16),
            "iota": iota_np,
            "wmat": w_np,
            "bcol": b_np,
        })
    return in_maps


def unshard(outs, node_lists):
    """outs: list of {'out': [128, NSLOT*128] f32} per core -> [50000,128]."""
    full = np.zeros((N_NODES, D), np.float32)
    for c in range(NCORES):
        oT = np.asarray(outs[c]["out"], np.float32)
        for s in range(NSLOT):
            ns = node_lists[c][s]
            if len(ns) == 0:
                continue
            full[ns, :] = oT[:, s * P:s * P + len(ns)].T
    return full


# ---------------------------------------------------------------- entry

_CACHE = {}


def kernel(features, src, dst, W, b):
    from concourse.bass_utils import run_bass_kernel_spmd

    pl = plan(src, dst)
    key = (tuple(pl["NB"]), tuple(pl["Klo"]), tuple(pl["Khi"]))
    if key not in _CACHE:
        _CACHE[key] = build(pl["NB"], pl["Klo"], pl["Khi"])
    nc = _CACHE[key]
    in_maps = make_in_maps(features, W, b, pl)
    last = None
    for _ in range(3):  # retry: a previously wedged pool device can fail a load
        try:
            res = run_bass_kernel_spmd(nc, in_maps, core_ids=list(range(NCORES)))
            return unshard(res.results, pl["node_lists"])
        except Exception as e:  # noqa: BLE001
            last = e
    raise last
